# revision 1
# baseline (speedup 1.0000x reference)
"""CircleLayer (histogram angle binning) Trainium2 Bass kernel.

Full-input contract: kernel(**inputs) takes the complete arrays, shards the
batch dim across 8 NeuronCores (pure data parallel), runs one SPMD Bass
program, and gathers the full [B, P, 2*D] output.

Per-core layout (512 samples = 4 tiles of 128):
  - geometry phase in [128 samples(part), 128 neighbors(free)] tiles on DVE/ACT
  - angle bins via exact fp32 thresholds (precomputed to match the reference's
    fp32 divide + int32 trunc semantics bit-exactly)
  - onehot (scaled by 1/n) transposed on PE, then 128 per-sample matmuls
    onehotS[128n, 8].T @ f_res[128n, 64] -> PSUM (16 samples packed per bank)
  - f_scan = relu(scan @ W_ce + b_ce) computed batched on DVE + one ACT relu
"""

import numpy as np

B, N, T, D = 4096, 128, 20, 64
P = 8
NCORES = 8
BC = B // NCORES  # samples per core
TILE = 128
NT = BC // TILE  # tiles per core

PI32 = np.float32(np.pi)
TWOPI32 = np.float32(2.0 * np.pi)
C32 = np.float32((2.0 * np.pi) / P)  # bin width as the reference computes it


def _bin_thresholds():
    """T[p] = smallest fp32 x >= 0 with int32(fp32(x / C32)) >= p.

    Comparing dir >= T[p] then reproduces the reference's
    (dir / C32).astype(int32) binning exactly (fp32 division is monotone).
    """
    thr = [np.float32(0.0)]
    for p in range(1, P + 1):
        x = np.float32(np.float32(p) * C32)
        while int(np.float32(x / C32)) >= p:
            x = np.nextafter(x, np.float32(-np.inf))
        while int(np.float32(x / C32)) < p:
            x = np.nextafter(x, np.float32(np.inf))
        thr.append(np.float32(x))
    return thr


THR = _bin_thresholds()

_prog_cache = {}


def _build_program():
    import concourse.bass as bass
    import concourse.tile as tile
    from concourse import bacc, mybir
    from concourse.masks import make_identity

    f32 = mybir.dt.float32
    AX = mybir.AxisListType
    OP = mybir.AluOpType
    AF = mybir.ActivationFunctionType

    nc = bacc.Bacc(
        "TRN2",
        target_bir_lowering=False,
        debug=False,
        enable_asserts=False,
        num_devices=NCORES,
    )

    nei = nc.dram_tensor("nei", [BC, N * T * 2], f32, kind="ExternalInput").ap()
    fresT = nc.dram_tensor("fresT", [N, BC * D], f32, kind="ExternalInput").ap()
    egoR = nc.dram_tensor("egoR", [TILE, NT * 2], f32, kind="ExternalInput").ap()
    wb = nc.dram_tensor("wb", [TILE, 3 * D], f32, kind="ExternalInput").ap()
    res_out = nc.dram_tensor("res_out", [128, NT * 512], f32, kind="ExternalOutput").ap()
    fscan_out = nc.dram_tensor("fscan_out", [BC, P * D], f32, kind="ExternalOutput").ap()

    FREE_NEI = N * T * 2  # 10240 floats per sample

    with tile.TileContext(nc) as tc:
        with (
            tc.tile_pool(name="const", bufs=1) as constp,
            tc.tile_pool(name="nei", bufs=2) as neip,
            tc.tile_pool(name="fres", bufs=2) as fresp,
            tc.tile_pool(name="geo", bufs=2) as geo,
            tc.tile_pool(name="small", bufs=2) as small,
            tc.tile_pool(name="oht", bufs=2) as ohtp,
            tc.tile_pool(name="tpsum", bufs=2, space="PSUM") as tpsum,
            tc.tile_pool(name="opsum", bufs=4, space="PSUM") as opsum,
        ):
            ident = constp.tile([128, 128], f32)
            make_identity(nc, ident[:])
            ego_sb = constp.tile([TILE, NT * 2], f32)
            nc.sync.dma_start(out=ego_sb[:], in_=egoR)
            wb_sb = constp.tile([TILE, 3 * D], f32)
            nc.sync.dma_start(out=wb_sb[:], in_=wb)
            w0 = wb_sb[:, 0:D]
            w1 = wb_sb[:, D : 2 * D]
            bias = wb_sb[:, 2 * D : 3 * D]

            for t in range(NT):
                rows = slice(t * TILE, (t + 1) * TILE)

                nei_sb = neip.tile([TILE, FREE_NEI], f32)
                nc.sync.dma_start(out=nei_sb[:], in_=nei[rows, :])
                nei_v = nei_sb[:].rearrange("b (n f) -> b n f", f=T * 2)

                fres_sb = fresp.tile([N, TILE * D], f32)
                nc.sync.dma_start(
                    out=fres_sb[:],
                    in_=fresT[:, t * TILE * D : (t + 1) * TILE * D],
                )

                # --- geometry ---
                msum = geo.tile([TILE, N], f32)
                nc.vector.tensor_reduce(msum[:], nei_v, axis=AX.X, op=OP.add)

                egox = ego_sb[:, 2 * t : 2 * t + 1]
                egoy = ego_sb[:, 2 * t + 1 : 2 * t + 2]
                relx = geo.tile([TILE, N], f32)
                nc.vector.tensor_scalar(relx[:], nei_v[:, :, 2 * T - 2], egox, None, OP.subtract)
                rely = geo.tile([TILE, N], f32)
                nc.vector.tensor_scalar(rely[:], nei_v[:, :, 2 * T - 1], egoy, None, OP.subtract)

                sqx = geo.tile([TILE, N], f32)
                nc.scalar.square(sqx[:], relx[:])
                sqy = geo.tile([TILE, N], f32)
                nc.scalar.square(sqy[:], rely[:])
                d2 = geo.tile([TILE, N], f32)
                nc.vector.tensor_tensor(d2[:], sqx[:], sqy[:], op=OP.add)
                dist = geo.tile([TILE, N], f32)
                nc.scalar.sqrt(dist[:], d2[:])

                # atan2(relx, rely): y=relx, x=rely.
                # ACT Arctan domain is [-pi/2, pi/2] -> octant reduction:
                # at_r = atan(min(|x|,|y|)/max(|x|,|y|)) in [0, pi/4]
                ax = geo.tile([TILE, N], f32)   # |y| = |relx|
                nc.scalar.activation(ax[:], relx[:], AF.Abs)
                ay = geo.tile([TILE, N], f32)   # |x| = |rely|
                nc.scalar.activation(ay[:], rely[:], AF.Abs)
                mn = geo.tile([TILE, N], f32)
                nc.vector.tensor_tensor(mn[:], ax[:], ay[:], op=OP.min)
                mx = geo.tile([TILE, N], f32)
                nc.vector.tensor_tensor(mx[:], ax[:], ay[:], op=OP.max)
                scr = geo.tile([TILE, N], f32)
                invmx = geo.tile([TILE, N], f32)
                nc.vector.reciprocal_approx_accurate(out=invmx[:], in_=mx[:], scratch=scr[:])
                qr = geo.tile([TILE, N], f32)
                nc.vector.tensor_tensor(qr[:], mn[:], invmx[:], op=OP.mult)
                atr = geo.tile([TILE, N], f32)
                nc.scalar.activation(atr[:], qr[:], AF.Arctan)

                # le = (|y| <= |x|): atan(|q|) = le ? atr : pi/2 - atr
                le = geo.tile([TILE, N], f32)
                nc.vector.tensor_tensor(le[:], ax[:], ay[:], op=OP.is_le)
                u1 = geo.tile([TILE, N], f32)
                nc.vector.tensor_scalar(u1[:], atr[:], -1.0, float(np.float32(np.pi / 2)), OP.mult, OP.add)
                dd = geo.tile([TILE, N], f32)
                nc.vector.tensor_tensor(dd[:], atr[:], u1[:], op=OP.subtract)
                m1 = geo.tile([TILE, N], f32)
                nc.vector.tensor_tensor(m1[:], le[:], dd[:], op=OP.mult)
                aq = geo.tile([TILE, N], f32)   # atan(|q|) in [0, pi/2]
                nc.vector.tensor_tensor(aq[:], u1[:], m1[:], op=OP.add)

                # theta_abs = atan2(|y|, x) = xlt ? pi - aq : aq
                xlt = geo.tile([TILE, N], f32)
                nc.gpsimd.tensor_scalar(xlt[:], rely[:], 0.0, None, OP.is_lt)
                t2 = geo.tile([TILE, N], f32)
                nc.vector.tensor_scalar(t2[:], aq[:], -2.0, float(PI32), OP.mult, OP.add)
                m2 = geo.tile([TILE, N], f32)
                nc.vector.tensor_tensor(m2[:], xlt[:], t2[:], op=OP.mult)
                th = geo.tile([TILE, N], f32)
                nc.vector.tensor_tensor(th[:], aq[:], m2[:], op=OP.add)

                # dir = mod(atan2(y,x), 2pi) = ylt ? 2pi - theta_abs : theta_abs
                ylt = geo.tile([TILE, N], f32)
                nc.gpsimd.tensor_scalar(ylt[:], relx[:], 0.0, None, OP.is_lt)
                t3 = geo.tile([TILE, N], f32)
                nc.vector.tensor_scalar(t3[:], th[:], -2.0, float(TWOPI32), OP.mult, OP.add)
                m3 = geo.tile([TILE, N], f32)
                nc.vector.tensor_tensor(m3[:], ylt[:], t3[:], op=OP.mult)
                dirw = geo.tile([TILE, N], f32)
                nc.vector.tensor_tensor(dirw[:], th[:], m3[:], op=OP.add)

                # invalid neighbors (all-zero traj) -> dirm = -10 -> no bin
                eq0 = geo.tile([TILE, N], f32)
                nc.gpsimd.tensor_scalar(eq0[:], msum[:], 0.0, None, OP.is_equal)
                tmsk = geo.tile([TILE, N], f32)
                nc.vector.scalar_tensor_tensor(
                    out=tmsk[:], in0=dirw[:], scalar=10.0, in1=eq0[:],
                    op0=OP.add, op1=OP.mult,
                )
                dirm = geo.tile([TILE, N], f32)
                nc.vector.tensor_tensor(dirm[:], dirw[:], tmsk[:], op=OP.subtract)

                # --- binning ---
                ges = []
                for p in range(P + 1):
                    gep = geo.tile([TILE, N], f32, tag=f"ge{p}")
                    nc.vector.tensor_scalar(gep[:], dirm[:], float(THR[p]), None, OP.is_ge)
                    ges.append(gep)

                nvec = small.tile([TILE, P], f32)
                ohs = []
                for p in range(P):
                    ohp = geo.tile([TILE, N], f32, tag=f"oh{p}")
                    nc.vector.scalar_tensor_tensor(
                        out=ohp[:], in0=ges[p][:], scalar=0.0, in1=ges[p + 1][:],
                        op0=OP.add, op1=OP.subtract,
                        accum_out=nvec[:, p : p + 1],
                    )
                    ohs.append(ohp)

                nadj = small.tile([TILE, P], f32)
                nc.vector.tensor_scalar(nadj[:], nvec[:], 1e-4, None, OP.add)
                invn = small.tile([TILE, P], f32)
                nc.vector.reciprocal(invn[:], nadj[:])

                mdist = small.tile([TILE, P], f32)
                mdir = small.tile([TILE, P], f32)
                ohT = ohtp.tile([N, P * TILE], f32)
                tps = []
                for p in range(P):
                    ohsp = geo.tile([TILE, N], f32, tag=f"ohs{p}")
                    nc.vector.tensor_scalar(ohsp[:], ohs[p][:], invn[:, p : p + 1], None, OP.mult)

                    scr2 = geo.tile([TILE, N], f32, tag="scr2")
                    nc.vector.scalar_tensor_tensor(
                        out=scr2[:], in0=dist[:], scalar=0.0, in1=ohsp[:],
                        op0=OP.add, op1=OP.mult, accum_out=mdist[:, p : p + 1],
                    )
                    scr3 = geo.tile([TILE, N], f32, tag="scr3")
                    nc.vector.scalar_tensor_tensor(
                        out=scr3[:], in0=dirw[:], scalar=0.0, in1=ohsp[:],
                        op0=OP.add, op1=OP.mult, accum_out=mdir[:, p : p + 1],
                    )

                    if p % 4 == 0:
                        tp = tpsum.tile([128, 512], f32, tag=f"tp{p // 4}")
                        tps.append(tp)
                    nc.tensor.transpose(
                        tp[:, (p % 4) * TILE : (p % 4 + 1) * TILE], ohsp[:], ident[:]
                    )

                nc.scalar.copy(ohT[:, 0:512], tps[0][:])
                nc.scalar.copy(ohT[:, 512:1024], tps[1][:])

                # --- f_scan = relu(scan @ W + b), batched over samples ---
                fpre = geo.tile([TILE, P * D], f32, tag="fpre")
                for p in range(P):
                    t1 = geo.tile([TILE, D], f32, tag="t1")
                    nc.vector.scalar_tensor_tensor(
                        out=t1[:], in0=w0, scalar=mdist[:, p : p + 1], in1=bias,
                        op0=OP.mult, op1=OP.add,
                    )
                    nc.vector.scalar_tensor_tensor(
                        out=fpre[:, p * D : (p + 1) * D], in0=w1,
                        scalar=mdir[:, p : p + 1], in1=t1[:],
                        op0=OP.mult, op1=OP.add,
                    )
                fscan = geo.tile([TILE, P * D], f32, tag="fscan")
                nc.scalar.activation(fscan[:], fpre[:], AF.Relu)

                nc.sync.dma_start(out=fscan_out[rows, :], in_=fscan[:])

                # --- per-sample binning matmuls (flipped): out = f_res[b].T @ onehotS[b]
                # [64 d, 8 p] per sample; 2 samples on partition strips {0,64},
                # 64 samples along free -> whole 128-sample tile in ONE bank ---
                ohT_v = ohT[:].rearrange("n (p b) -> n p b", b=TILE)
                pres = opsum.tile([128, 512], f32, tag="pres")
                for s in range(TILE):
                    s2, s64 = s % 2, s // 2
                    nc.tensor.matmul(
                        pres[s2 * 64 : (s2 + 1) * 64, s64 * P : (s64 + 1) * P],
                        fres_sb[:, s * D : (s + 1) * D],
                        ohT_v[:, :, s],
                        start=True,
                        stop=True,
                    )
                stage = geo.tile([128, 512], f32, tag="stage")
                nc.scalar.copy(stage[:], pres[:])
                nc.sync.dma_start(
                    out=res_out[:, t * 512 : (t + 1) * 512], in_=stage[:]
                )

    nc.compile()
    return nc


def _get_program():
    if "nc" not in _prog_cache:
        _prog_cache["nc"] = _build_program()
    return _prog_cache["nc"]


def kernel(ego_traj_2d, nei_traj_2d, f_resonance, W_ce, b_ce):
    from concourse import bass_utils

    ego_traj_2d = np.asarray(ego_traj_2d, dtype=np.float32)
    nei_traj_2d = np.asarray(nei_traj_2d, dtype=np.float32)
    f_resonance = np.asarray(f_resonance, dtype=np.float32)
    W_ce = np.asarray(W_ce, dtype=np.float32)
    b_ce = np.asarray(b_ce, dtype=np.float32)

    nc = _get_program()

    wb_full = np.empty((TILE, 3 * D), dtype=np.float32)
    wb_full[:, 0:D] = W_ce[0]
    wb_full[:, D : 2 * D] = W_ce[1]
    wb_full[:, 2 * D : 3 * D] = b_ce

    ego_last = ego_traj_2d[:, -1, :]  # [B, 2]

    in_maps = []
    for c in range(NCORES):
        rows = slice(c * BC, (c + 1) * BC)
        nei_c = nei_traj_2d[rows].reshape(BC, N * T * 2)
        fresT_c = np.ascontiguousarray(
            f_resonance[rows].transpose(1, 0, 2)
        ).reshape(N, BC * D)
        egoR_c = np.ascontiguousarray(
            ego_last[rows].reshape(NT, TILE, 2).transpose(1, 0, 2)
        ).reshape(TILE, NT * 2)
        in_maps.append(
            {
                "nei": np.ascontiguousarray(nei_c),
                "fresT": fresT_c,
                "egoR": egoR_c,
                "wb": wb_full,
            }
        )

    res = bass_utils.run_bass_kernel_spmd(nc, in_maps, core_ids=list(range(NCORES)))
    outs = [
        decode_core(res.results[c]["res_out"], res.results[c]["fscan_out"])
        for c in range(NCORES)
    ]
    return np.concatenate(outs, axis=0)


def decode_core(res_raw, fscan_raw):
    """res_out row q = s2*64 + d, col = t*512 + s64*8 + p, sample b = t*128 + s64*2 + s2."""
    r = res_raw.reshape(2, D, NT, 64, P).transpose(2, 3, 0, 4, 1).reshape(BC, P, D)
    f = fscan_raw.reshape(BC, P, D)
    return np.concatenate([r, f], axis=-1)

